# revision 1
# baseline (speedup 1.0000x reference)
"""Heat equation (512x512, 399 output steps) on 8 trn2 NeuronCores.

Sharding: 1D row strips, 64 owned rows/core, 32-row deep halo each side
(tile = 128 partitions x 512 cols). Halo refreshed via AllGather every 32
steps; rank +/-1 slices selected with dynamic-offset DMAs (OOB -> skip on
edge cores). Update: T' = T + A o (up+dn+lt+rt-4c), A = DT*dmap/DX2 with
A=0 on global boundary (preserves Dirichlet zeros). Step 1 additionally
masks the carried-over u0 boundary rows to zero.
"""
import numpy as np

N = 512
NCORES = 8
ROWS = 64          # owned rows per core
H = 32             # halo depth
STEPS = 399        # compute T_1..T_399
PERIOD = 32        # halo exchange period
DT = 5e-7
DX = 1.0 / (N - 1)
DX2 = DX * DX
PX = PY = 16

_prog_cache = {}


def _build_program(steps, period):
    from concourse import bass, bacc, tile, mybir

    DTf = mybir.dt.float32
    nc = bacc.Bacc("TRN2", target_bir_lowering=False, debug=False,
                   num_devices=NCORES)

    DTr = mybir.dt.float32r
    u0_in = nc.dram_tensor("u0t", [128, N + 2], DTf, kind="ExternalInput")
    zg_in = nc.dram_tensor("zguard", [128, 2], DTf, kind="ExternalInput")
    a_in = nc.dram_tensor("amap", [128, N], DTf, kind="ExternalInput")
    m_in = nc.dram_tensor("mask", [128, N], DTf, kind="ExternalInput")
    w_in = nc.dram_tensor("wud", [128, 128], DTf, kind="ExternalInput")
    i_in = nc.dram_tensor("ident", [128, 128], DTf, kind="ExternalInput")
    out = nc.dram_tensor("out", [steps, ROWS, N], DTf, kind="ExternalOutput")
    in_bounce = nc.dram_tensor("in_bounce", [ROWS, N], DTf)
    ag_out = nc.dram_tensor("ag_out", [NCORES * ROWS, N], DTf,
                            addr_space="Shared")

    add = mybir.AluOpType.add
    mult = mybir.AluOpType.mult

    with tile.TileContext(nc) as tc:
        with tc.tile_pool(name="state", bufs=1) as spool, \
             tc.tile_pool(name="consts", bufs=1) as cpool, \
             tc.tile_pool(name="psum", bufs=2, space="PSUM") as ppool, \
             tc.tile_pool(name="scratch", bufs=2) as zpool:
            st = [spool.tile([128, N + 2], DTf, tag=f"st{i}", name=f"st{i}")
                  for i in range(4)]
            amap = cpool.tile([128, N], DTf, tag="amap")
            mask = cpool.tile([128, N], DTf, tag="mask")
            wud = cpool.tile([128, 128], DTf, tag="wud")
            ident = cpool.tile([128, 128], DTf, tag="ident")

            nc.sync.dma_start(out=amap[:], in_=a_in[:])
            nc.sync.dma_start(out=mask[:], in_=m_in[:])
            nc.gpsimd.dma_start(out=wud[:].bitcast(DTr), in_=w_in[:])
            nc.gpsimd.dma_start(out=ident[:].bitcast(DTr), in_=i_in[:])
            nc.gpsimd.dma_start(out=st[0][:].bitcast(DTr), in_=u0_in[:])
            for i in range(1, 4):
                nc.gpsimd.dma_start(out=st[i][:, 0:1].bitcast(DTr),
                                    in_=zg_in[:, 0:1])
                nc.gpsimd.dma_start(out=st[i][:, N + 1:N + 2].bitcast(DTr),
                                    in_=zg_in[:, 1:2])

            # rank-dependent AllGather read offsets (computed once)
            r = nc.gpsimd.partition_id()
            ofs_top = nc.s_assert_within(r * ROWS - H, 0, NCORES * ROWS - H,
                                         skip_runtime_assert=True)
            ofs_bot = nc.s_assert_within(r * ROWS + ROWS, 0,
                                         NCORES * ROWS - H,
                                         skip_runtime_assert=True)

            dma_engines = [nc.sync, nc.gpsimd, nc.scalar]

            for t in range(1, steps + 1):
                Tp = st[(t - 1) % 4]
                Tn = st[t % 4]
                pl = ppool.tile([128, N], DTf, tag="pl")
                m4 = zpool.tile([128, N], DTf, tag="m4")
                # 5-point laplacian sum into PSUM:
                #   pl = up + dn - 4*c  (tridiag weights, partition dim)
                #   pl += lt ; pl += rt (shifted-identity, free dim)
                nc.tensor.matmul(pl[:], wud[:].bitcast(DTr),
                                 Tp[:, 1:N + 1].bitcast(DTr),
                                 start=True, stop=False)
                nc.tensor.matmul(pl[:], ident[:].bitcast(DTr),
                                 Tp[:, 0:N].bitcast(DTr),
                                 start=False, stop=False,
                                 skip_group_check=True)
                nc.tensor.matmul(pl[:], ident[:].bitcast(DTr),
                                 Tp[:, 2:N + 2].bitcast(DTr),
                                 start=False, stop=True,
                                 skip_group_check=True)
                nc.vector.tensor_tensor(m4[:], amap[:], pl[:], mult)
                if t == 1:
                    tm = zpool.tile([128, N], DTf, tag="tm")
                    nc.vector.tensor_tensor(tm[:], Tp[:, 1:N + 1],
                                            mask[:], mult)
                    nc.vector.tensor_tensor(Tn[:, 1:N + 1].bitcast(DTr),
                                            tm[:], m4[:], add)
                else:
                    nc.vector.tensor_tensor(Tn[:, 1:N + 1].bitcast(DTr),
                                            Tp[:, 1:N + 1], m4[:], add)

                eng = dma_engines[t % len(dma_engines)]
                eng.dma_start(out=out[t - 1], in_=Tn[32:96, 1:N + 1])

                if t % period == 0 and t < steps:
                    nc.sync.dma_start(out=in_bounce[:], in_=Tn[32:96, 1:N + 1])
                    nc.gpsimd.collective_compute(
                        "AllGather",
                        mybir.AluOpType.bypass,
                        replica_groups=[list(range(NCORES))],
                        ins=[in_bounce[:]],
                        outs=[ag_out[:]],
                    )
                    nc.gpsimd.dma_start(out=Tn[0:H, 1:N + 1].bitcast(DTr),
                                        in_=ag_out[bass.ds(ofs_top, H), :],
                                        bounds_check="skip_entire_dma")
                    nc.gpsimd.dma_start(out=Tn[96:128, 1:N + 1].bitcast(DTr),
                                        in_=ag_out[bass.ds(ofs_bot, H), :],
                                        bounds_check="skip_entire_dma")

    nc.compile()
    return nc


def _bilinear_f32(a, out_h, out_w):
    """numpy float32 mirror of reference.bilinear_align_corners."""
    in_h, in_w = a.shape
    ys = np.linspace(0.0, in_h - 1.0, out_h, dtype=np.float32)
    xs = np.linspace(0.0, in_w - 1.0, out_w, dtype=np.float32)
    y0 = np.clip(np.floor(ys).astype(np.int32), 0, in_h - 2)
    x0 = np.clip(np.floor(xs).astype(np.int32), 0, in_w - 2)
    wy = (ys - y0.astype(np.float32))[:, None]
    wx = (xs - x0.astype(np.float32))[None, :]
    a00 = a[y0][:, x0]
    a01 = a[y0][:, x0 + 1]
    a10 = a[y0 + 1][:, x0]
    a11 = a[y0 + 1][:, x0 + 1]
    return (a00 * (1 - wy) * (1 - wx) + a01 * (1 - wy) * wx
            + a10 * wy * (1 - wx) + a11 * wy * wx).astype(np.float32)


def kernel(u0, alpha, steps=STEPS, period=PERIOD):
    from concourse.bass_utils import run_bass_kernel_spmd

    u0 = np.asarray(u0, dtype=np.float32)
    alpha = np.asarray(alpha, dtype=np.float32)

    dmap = _bilinear_f32(alpha, N, N)
    A = (np.float32(DT) * dmap / np.float32(DX2)).astype(np.float32)
    A[0, :] = 0.0
    A[N - 1, :] = 0.0
    A[:, 0] = 0.0
    A[:, N - 1] = 0.0

    in_maps = []
    for i in range(NCORES):
        lo = i * ROWS - H          # global row of tile partition 0
        u0t = np.zeros((128, N + 2), np.float32)
        at = np.zeros((128, N), np.float32)
        g0, g1 = max(lo, 0), min(lo + 128, N)
        u0t[g0 - lo:g1 - lo, 1:N + 1] = u0[g0:g1]
        at[g0 - lo:g1 - lo] = A[g0:g1]
        mt = np.ones((128, N), np.float32)
        mt[:, 0] = 0.0
        mt[:, N - 1] = 0.0
        if i == 0:
            mt[H] = 0.0            # global row 0 at partition 32
        if i == NCORES - 1:
            mt[H + ROWS - 1] = 0.0  # global row 511 at partition 95
        wud = np.zeros((128, 128), np.float32)
        for m in range(128):
            wud[m, m] = -4.0
            if m > 0:
                wud[m - 1, m] = 1.0
            if m < 127:
                wud[m + 1, m] = 1.0
        ident = np.eye(128, dtype=np.float32)
        in_maps.append({"u0t": u0t, "amap": at, "mask": mt,
                        "wud": wud, "ident": ident,
                        "zguard": np.zeros((128, 2), np.float32)})

    key = (steps, period)
    if key not in _prog_cache:
        _prog_cache[key] = _build_program(steps, period)
    nc = _prog_cache[key]

    res = run_bass_kernel_spmd(nc, in_maps, list(range(NCORES)))
    globals()["_last_res"] = res
    full = np.concatenate([res.results[i]["out"] for i in range(NCORES)],
                          axis=1)
    return full



# revision 3
# speedup vs baseline: 1.5433x; 1.5433x over previous
"""Heat equation (512x512, 399 output steps) on 8 trn2 NeuronCores.

v2: fp16 state/compute. Host computes step 1 (f32); device computes steps
2..399. Sharding: 1D row strips, 64 owned rows/core, 32-row halo each side
(tile = 128 partitions x 514 cols fp16). Halo refreshed via fp16 AllGather
every 32 steps.

Per step, split into L/R column halves for cross-engine pipelining:
  PE : psum = wud@Tc + ident@Tl + ident@Tr   (5-point laplacian, f32 PSUM)
  Act: c = Copy(psum) -> fp16 SBUF
  DVE: m = A o c ; Tn = Tp + m               (fp16 2x passes)
A = DT*dmap/DX2 (fp16, zeroed on the global boundary) keeps Dirichlet rows
/cols frozen at zero.
"""
import numpy as np

N = 512
NCORES = 8
ROWS = 64          # owned rows per core
H = 32             # halo depth
DSTEPS = 398       # device computes T_2..T_399
PERIOD = 32        # halo exchange period
DT = 5e-7
DX = 1.0 / (N - 1)
DX2 = DX * DX

_prog_cache = {}


def _build_program(steps, period):
    from concourse import bass, bacc, tile, mybir

    F16 = mybir.dt.float16
    F32 = mybir.dt.float32
    nc = bacc.Bacc("TRN2", target_bir_lowering=False, debug=False,
                   num_devices=NCORES)

    t1_in = nc.dram_tensor("t1t", [128, N + 2], F16, kind="ExternalInput")
    a_in = nc.dram_tensor("amap", [128, N], F16, kind="ExternalInput")
    w_in = nc.dram_tensor("wud", [128, 128], F16, kind="ExternalInput")
    i_in = nc.dram_tensor("ident", [128, 128], F16, kind="ExternalInput")
    z_in = nc.dram_tensor("zg", [128, 2], F16, kind="ExternalInput")
    out = nc.dram_tensor("out", [steps, ROWS, N], F16, kind="ExternalOutput")
    in_bounce = nc.dram_tensor("in_bounce", [ROWS, N], F16)
    ag_out = nc.dram_tensor("ag_out", [NCORES * ROWS, N], F16,
                            addr_space="Shared")

    add = mybir.AluOpType.add
    mult = mybir.AluOpType.mult
    COPY = mybir.ActivationFunctionType.Copy
    HALF = N // 2

    with tile.TileContext(nc) as tc:
        with tc.tile_pool(name="state", bufs=1) as spool, \
             tc.tile_pool(name="consts", bufs=1) as cpool, \
             tc.tile_pool(name="psum", bufs=4, space="PSUM") as ppool, \
             tc.tile_pool(name="scratch", bufs=3) as zpool:
            st = [spool.tile([128, N + 2], F16, tag=f"st{i}", name=f"st{i}")
                  for i in range(4)]
            amap = cpool.tile([128, N], F16, tag="amap")
            wud = cpool.tile([128, 128], F16, tag="wud")
            ident = cpool.tile([128, 128], F16, tag="ident")

            nc.sync.dma_start(out=amap[:], in_=a_in[:])
            nc.sync.dma_start(out=wud[:], in_=w_in[:])
            nc.sync.dma_start(out=ident[:], in_=i_in[:])
            nc.sync.dma_start(out=st[0][:], in_=t1_in[:])
            for i in range(1, 4):
                nc.gpsimd.dma_start(out=st[i][:, 0:1], in_=z_in[:, 0:1])
                nc.gpsimd.dma_start(out=st[i][:, N + 1:N + 2], in_=z_in[:, 1:2])

            # rank-dependent AllGather read offsets (computed once)
            r = nc.gpsimd.partition_id()
            ofs_top = nc.s_assert_within(r * ROWS - H, 0, NCORES * ROWS - H,
                                         skip_runtime_assert=True)
            ofs_bot = nc.s_assert_within(r * ROWS + ROWS, 0,
                                         NCORES * ROWS - H,
                                         skip_runtime_assert=True)

            dma_engines = [nc.sync, nc.gpsimd, nc.scalar]

            for k in range(steps):
                Tp = st[k % 4]
                Tn = st[(k + 1) % 4]
                psL = ppool.tile([128, HALF], F32, tag="psL")
                psR = ppool.tile([128, HALF], F32, tag="psR")
                cL = zpool.tile([128, HALF], F16, tag="cL")
                cR = zpool.tile([128, HALF], F16, tag="cR")
                mL = zpool.tile([128, HALF], F16, tag="mL")
                mR = zpool.tile([128, HALF], F16, tag="mR")

                # laplacian into PSUM per half; weight order minimizes
                # LDWEIGHTS swaps: wud (cL,cR) then ident (ltL,rtL,ltR,rtR)
                nc.tensor.matmul(psL[:], wud[:], Tp[:, 1:HALF + 1],
                                 start=True, stop=False)
                nc.tensor.matmul(psR[:], wud[:], Tp[:, HALF + 1:N + 1],
                                 start=True, stop=False)
                nc.tensor.matmul(psL[:], ident[:], Tp[:, 0:HALF],
                                 start=False, stop=False, skip_group_check=True)
                nc.tensor.matmul(psL[:], ident[:], Tp[:, 2:HALF + 2],
                                 start=False, stop=True, skip_group_check=True)
                nc.tensor.matmul(psR[:], ident[:], Tp[:, HALF:N],
                                 start=False, stop=False, skip_group_check=True)
                nc.tensor.matmul(psR[:], ident[:], Tp[:, HALF + 2:N + 2],
                                 start=False, stop=True, skip_group_check=True)

                # PSUM -> fp16 SBUF on the scalar (Act) engine
                nc.scalar.activation(cL[:], psL[:], COPY)
                nc.scalar.activation(cR[:], psR[:], COPY)

                # m = A o c ; Tn = Tp + m   (DVE, fp16 2x)
                nc.vector.tensor_tensor(mL[:], amap[:, 0:HALF], cL[:], mult)
                nc.vector.tensor_tensor(Tn[:, 1:HALF + 1],
                                        Tp[:, 1:HALF + 1], mL[:], add)
                nc.vector.tensor_tensor(mR[:], amap[:, HALF:N], cR[:], mult)
                nc.vector.tensor_tensor(Tn[:, HALF + 1:N + 1],
                                        Tp[:, HALF + 1:N + 1], mR[:], add)

                eng = dma_engines[k % len(dma_engines)]
                eng.dma_start(out=out[k], in_=Tn[H:H + ROWS, 1:N + 1])

                if (k + 2) % period == 0 and k < steps - 14:
                    # refresh halos with an AllGather of the owned rows
                    nc.sync.dma_start(out=in_bounce[:],
                                      in_=Tn[H:H + ROWS, 1:N + 1])
                    nc.gpsimd.collective_compute(
                        "AllGather",
                        mybir.AluOpType.bypass,
                        replica_groups=[list(range(NCORES))],
                        ins=[in_bounce[:]],
                        outs=[ag_out[:]],
                    )
                    nc.gpsimd.dma_start(out=Tn[0:H, 1:N + 1],
                                        in_=ag_out[bass.ds(ofs_top, H), :],
                                        bounds_check="skip_entire_dma")
                    nc.gpsimd.dma_start(out=Tn[H + ROWS:128, 1:N + 1],
                                        in_=ag_out[bass.ds(ofs_bot, H), :],
                                        bounds_check="skip_entire_dma")

    nc.compile()
    return nc


def _bilinear_f32(a, out_h, out_w):
    """numpy float32 mirror of reference bilinear_align_corners."""
    in_h, in_w = a.shape
    ys = np.linspace(0.0, in_h - 1.0, out_h, dtype=np.float32)
    xs = np.linspace(0.0, in_w - 1.0, out_w, dtype=np.float32)
    y0 = np.clip(np.floor(ys).astype(np.int32), 0, in_h - 2)
    x0 = np.clip(np.floor(xs).astype(np.int32), 0, in_w - 2)
    wy = (ys - y0.astype(np.float32))[:, None]
    wx = (xs - x0.astype(np.float32))[None, :]
    a00 = a[y0][:, x0]
    a01 = a[y0][:, x0 + 1]
    a10 = a[y0 + 1][:, x0]
    a11 = a[y0 + 1][:, x0 + 1]
    return (a00 * (1 - wy) * (1 - wx) + a01 * (1 - wy) * wx
            + a10 * wy * (1 - wx) + a11 * wy * wx).astype(np.float32)


def kernel(u0, alpha, steps=DSTEPS, period=PERIOD):
    from concourse.bass_utils import run_bass_kernel_spmd

    u0 = np.asarray(u0, dtype=np.float32)
    alpha = np.asarray(alpha, dtype=np.float32)

    dmap = _bilinear_f32(alpha, N, N)
    a_in = dmap[1:-1, 1:-1]

    # host computes step 1 exactly as the f32 reference does
    lap = (u0[:-2, 1:-1] - 2.0 * u0[1:-1, 1:-1] + u0[2:, 1:-1]
           + u0[1:-1, :-2] - 2.0 * u0[1:-1, 1:-1] + u0[1:-1, 2:]) / np.float32(DX2)
    inner = u0[1:-1, 1:-1] + np.float32(DT) * a_in * lap
    T1 = np.zeros((N, N), np.float32)
    T1[1:-1, 1:-1] = inner

    A = (np.float32(DT) * dmap / np.float32(DX2)).astype(np.float32)
    A[0, :] = 0.0
    A[N - 1, :] = 0.0
    A[:, 0] = 0.0
    A[:, N - 1] = 0.0

    T1h = T1.astype(np.float16)
    Ah = A.astype(np.float16)

    wud = np.zeros((128, 128), np.float16)
    for m in range(128):
        wud[m, m] = -4.0
        if m > 0:
            wud[m - 1, m] = 1.0
        if m < 127:
            wud[m + 1, m] = 1.0
    ident = np.eye(128, dtype=np.float16)

    in_maps = []
    for i in range(NCORES):
        lo = i * ROWS - H          # global row of tile partition 0
        t1t = np.zeros((128, N + 2), np.float16)
        at = np.zeros((128, N), np.float16)
        g0, g1 = max(lo, 0), min(lo + 128, N)
        t1t[g0 - lo:g1 - lo, 1:N + 1] = T1h[g0:g1]
        at[g0 - lo:g1 - lo] = Ah[g0:g1]
        in_maps.append({"t1t": t1t, "amap": at, "wud": wud, "ident": ident,
                        "zg": np.zeros((128, 2), np.float16)})

    key = (steps, period)
    if key not in _prog_cache:
        _prog_cache[key] = _build_program(steps, period)
    nc = _prog_cache[key]

    res = run_bass_kernel_spmd(nc, in_maps, list(range(NCORES)))
    globals()["_last_res"] = res
    full = np.empty((steps + 1, N, N), np.float32)
    full[0] = T1
    dev = np.concatenate([res.results[i]["out"] for i in range(NCORES)],
                         axis=1)
    full[1:] = dev.astype(np.float32)
    return full
